# revision 17
# baseline (speedup 1.0000x reference)
"""Trainium2 Bass kernel for DAG sparse self-attention block.

Per-core layout (data-parallel over batch, 1 batch / core):
  obs/act (1024,256) f32, mask (1024,1024) i32 -> out (1024,256) f32.

v2 design (vs v1 baseline):
  - mask applied multiplicatively AFTER exp (e = exp(s) * m), split across
    DVE and GpSimd -- removes 128 identity matmuls from the PE.
  - scores: single K=32 matmul per PSUM bank (no mask accumulation),
    row groups via tile_position.
  - attn@v: v is the stationary ([v_a|1|0*31] 64 cols / [v_b|1|0*31]) and
    the big e tiles stream as the moving operand; heads col-packed in
    64-col groups.  Softmax denominators fall out at rows 32/96.
  - y normalized feature-major: reciprocal_approx_fast on the denom rows,
    DRAM-roundtrip partition-broadcast, one TT per pair; results land as
    feature-major z chunks (head order permuted, weights permuted to match).
  - LN1 folded into the projection weights host-side (W1 = Wp*g1; rank-1
    corrections -mean1 (x) c1 and (1/rstd1) (x) c2 via K=1 matmuls); the
    per-token rstd1 scale rides the ACT Gelu `scale` operand.
  - all rsqrt/recip via Ln/Exp (natural_log_exp table set): 4 ACT table
    loads total (gelu, exp, gelu, exp).
"""

import numpy as np

P = 128
L = 1024
D = 256
DD = 512
H = 8
HD = 32
NLB = L // P
NMB = L // P
NCORES = 8
EPS = 1e-5

# on-chip head order of the y-part features of z (pairs (0,1),(2,3),... ;
# even pair -> chunk rows {0:32,64:96}, odd pair -> {32:64,96:128})
Y_HEAD_ORDER = [2, 0, 3, 1, 6, 4, 7, 5]

_CACHE = {}


def _build():
    import concourse.bass as bass
    import concourse.tile as tile
    from concourse import bacc, mybir

    f32 = mybir.dt.float32
    bf16 = mybir.dt.bfloat16
    AF = mybir.ActivationFunctionType
    ALU = mybir.AluOpType

    nc = bacc.Bacc()

    obs_bf = nc.declare_dram_parameter("obs_bf", [L, D], bf16, isOutput=False)
    act_bf = nc.declare_dram_parameter("act_bf", [L, D], bf16, isOutput=False)
    msk_bf = nc.declare_dram_parameter("msk_bf", [L, L], bf16, isOutput=False)
    wq_bf = nc.declare_dram_parameter("wq_bf", [D, D], bf16, isOutput=False)
    wk_bf = nc.declare_dram_parameter("wk_bf", [D, DD], bf16, isOutput=False)
    wv_bf = nc.declare_dram_parameter("wv_bf", [D, DD], bf16, isOutput=False)
    wobs_bf = nc.declare_dram_parameter("wobs_bf", [D, D], bf16, isOutput=False)
    w1t_bf = nc.declare_dram_parameter("w1t_bf", [DD, D], bf16, isOutput=False)
    bq = nc.declare_dram_parameter("bq", [D], f32, isOutput=False)
    bk = nc.declare_dram_parameter("bk", [D], f32, isOutput=False)
    bv_row = nc.declare_dram_parameter("bv_row", [1, D], bf16, isOutput=False)
    bobs = nc.declare_dram_parameter("bobs", [D], f32, isOutput=False)
    g_obs = nc.declare_dram_parameter("g_obs", [D], f32, isOutput=False)
    b_obs = nc.declare_dram_parameter("b_obs", [D], f32, isOutput=False)
    negc1_row = nc.declare_dram_parameter("negc1_row", [1, D], bf16, isOutput=False)
    c2_row = nc.declare_dram_parameter("c2_row", [1, D], bf16, isOutput=False)
    g2 = nc.declare_dram_parameter("g2", [D], f32, isOutput=False)
    b2 = nc.declare_dram_parameter("b2", [D], f32, isOutput=False)
    out = nc.declare_dram_parameter("out", [L, D], f32, isOutput=True)

    def dram_bcast(ap1d, n):
        return bass.AP(tensor=ap1d.tensor, offset=ap1d.offset, ap=[[0, n]] + ap1d.ap)

    with tile.TileContext(nc) as tc:
        with (
            tc.tile_pool(name="consts", bufs=1) as consts,
            tc.tile_pool(name="epool", bufs=36) as epool,
            tc.tile_pool(name="tmp", bufs=2) as tmp,
            tc.tile_pool(name="stg", bufs=2) as stg,
            tc.tile_pool(name="zsqp", bufs=2) as zsqp,
            tc.tile_pool(name="rrp", bufs=1) as rrp,
            tc.tile_pool(name="rbp", bufs=2) as rbp,
            tc.tile_pool(name="small", bufs=4) as small,
            tc.tile_pool(name="q2p", bufs=8) as q2p,
            tc.tile_pool(name="outp", bufs=2) as outp,
            tc.tile_pool(name="dpool", bufs=1, space="DRAM") as dpool,
            tc.tile_pool(name="ps", bufs=2, space="PSUM") as psum,
            tc.tile_pool(name="psy", bufs=2, space="PSUM") as psumy,
        ):
            # ---------- DMA transposed loads (sync queue, mask last) ------
            def dma_T(dst, src2d):
                nc.sync.dma_start(out=dst, in_=src2d, transpose=True)

            obsT = []
            actT = []
            for c in range(2):
                t = consts.tile([P, L], bf16, tag=f"obsT{c}", name=f"obsT{c}")
                dma_T(t[:], obs_bf[:, c * P:(c + 1) * P])
                obsT.append(t)
            for c in range(2):
                t = consts.tile([P, L], bf16, tag=f"actT{c}", name=f"actT{c}")
                dma_T(t[:], act_bf[:, c * P:(c + 1) * P])
                actT.append(t)
            augT = obsT + actT

            def load_wT(src, ncols, name):
                ts_ = []
                for c in range(ncols // P):
                    t = consts.tile([P, src.shape[0]], bf16, tag=f"{name}{c}", name=f"{name}{c}")
                    dma_T(t[:], src[:, c * P:(c + 1) * P])
                    ts_.append(t)
                return ts_

            wobsT = load_wT(wobs_bf, D, "wobsT")
            wqT = load_wT(wq_bf, D, "wqT")
            wkT = load_wT(wk_bf, DD, "wkT")
            wvT = load_wT(wv_bf, DD, "wvT")

            maskT = []
            for mb in range(NMB):
                t = consts.tile([P, L], bf16, tag=f"maskT{mb}", name=f"maskT{mb}")
                dma_T(t[:], msk_bf[:, mb * P:(mb + 1) * P])
                maskT.append(t)

            # ---------- plain DMA loads (gpsimd queue, parallel) ----------
            w1T = []
            for c in range(4):
                t = consts.tile([P, D], bf16, tag=f"w1T{c}", name=f"w1T{c}")
                nc.gpsimd.dma_start(out=t[:], in_=w1t_bf[c * P:(c + 1) * P, :])
                w1T.append(t)

            def load_col(src, c, name):
                t = consts.tile([P, 1], f32, tag=name, name=name)
                nc.gpsimd.dma_start(out=t[:], in_=src[c * P:(c + 1) * P])
                return t

            bq_c = [load_col(bq, c, f"bq{c}") for c in range(2)]
            bk_c = [load_col(bk, c, f"bk{c}") for c in range(2)]
            bobs_c = [load_col(bobs, c, f"bobs{c}") for c in range(2)]
            gobs_c = [load_col(g_obs, c, f"gobs{c}") for c in range(2)]
            bobsln_c = [load_col(b_obs, c, f"bobsln{c}") for c in range(2)]

            def load_row(src, name):
                t = consts.tile([1, D], bf16, tag=name, name=name)
                nc.gpsimd.dma_start(out=t[:], in_=src[:])
                return t

            negc1 = load_row(negc1_row, "negc1")
            c2r = load_row(c2_row, "c2r")
            bvr = load_row(bv_row, "bvr")

            g2b = consts.tile([P, D], bf16, tag="g2b", name="g2b")
            nc.gpsimd.dma_start(out=g2b[:], in_=dram_bcast(g2[:], P))
            b2b = consts.tile([P, D], f32, tag="b2b", name="b2b")
            nc.gpsimd.dma_start(out=b2b[:], in_=dram_bcast(b2[:], P))

            eps_t = consts.tile([P, 1], f32, tag="eps_t", name="eps_t")
            nc.vector.memset(eps_t[:], EPS)
            onesrow = consts.tile([1, P], bf16, tag="onesrow", name="onesrow")
            nc.vector.memset(onesrow[:], 1.0)
            inv512c = consts.tile([P, 1], bf16, tag="inv512c", name="inv512c")
            nc.vector.memset(inv512c[:], 1.0 / DD)
            inv256c = consts.tile([P, 1], bf16, tag="inv256c", name="inv256c")
            nc.vector.memset(inv256c[:], 1.0 / D)

            # v_aug pair stationaries [128 m, 128]: [v_a|1|0*31 | v_b|1|0*31]
            # va0: mc=0 head-A full-width variant with right half all-zero.
            va = []
            for mc in range(NMB):
                row = []
                for p in range(4):
                    t = consts.tile([P, 2 * P], bf16, tag=f"va{mc}_{p}", name=f"va{mc}_{p}")
                    nc.vector.memset(t[:], 0.0)
                    nc.vector.memset(t[:, 0:1], 1.0)
                    nc.vector.memset(t[:, P + 1:P + 2], 1.0)
                    row.append(t)
                va.append(row)

            # DRAM scratch
            mg_d = dpool.tile([1, L], f32, tag="mg_d", name="mg_d")
            rg_d = dpool.tile([1, L], f32, tag="rg_d", name="rg_d")
            r1_d = dpool.tile([1, L], f32, tag="r1_d", name="r1_d")
            rden_d = [dpool.tile([2, L], f32, tag=f"rden_d{p}", name=f"rden_d{p}")
                      for p in range(4)]

            # ---------- obs2 pre-projection + gelu (gelu set, early) ------
            g_t = []
            for oc in range(2):
                ps = psum.tile([P, L], f32, tag="ps", name=f"g_ps{oc}")
                for cc in range(2):
                    for nb in range(2):
                        nc.tensor.matmul(
                            ps[:, nb * DD:(nb + 1) * DD],
                            lhsT=wobsT[cc][:, oc * P:(oc + 1) * P],
                            rhs=obsT[cc][:, nb * DD:(nb + 1) * DD],
                            start=(cc == 0), stop=(cc == 1),
                        )
                gt = consts.tile([P, L], bf16, tag=f"g{oc}", name=f"g{oc}")
                nc.scalar.activation(gt[:], ps[:], AF.Gelu, bias=bobs_c[oc][:])
                g_t.append(gt)

            # ---------- q/k projections (feature-major bf16) --------------
            qT = []
            kT = []
            for dc in range(2):
                ps = psum.tile([P, L], f32, tag="ps", name=f"q_ps{dc}")
                for cc in range(2):
                    for nb in range(2):
                        nc.tensor.matmul(
                            ps[:, nb * DD:(nb + 1) * DD],
                            lhsT=wqT[cc][:, dc * P:(dc + 1) * P],
                            rhs=obsT[cc][:, nb * DD:(nb + 1) * DD],
                            start=(cc == 0), stop=(cc == 1),
                        )
                t = consts.tile([P, L], bf16, tag=f"qT{dc}", name=f"qT{dc}")
                nc.vector.tensor_scalar_add(t[:], in0=ps[:], scalar1=bq_c[dc][:])
                qT.append(t)
            for dc in range(2):
                ps = psum.tile([P, L], f32, tag="ps", name=f"k_ps{dc}")
                for cc in range(4):
                    for nb in range(2):
                        nc.tensor.matmul(
                            ps[:, nb * DD:(nb + 1) * DD],
                            lhsT=wkT[cc][:, dc * P:(dc + 1) * P],
                            rhs=augT[cc][:, nb * DD:(nb + 1) * DD],
                            start=(cc == 0), stop=(cc == 3),
                        )
                t = consts.tile([P, L], bf16, tag=f"kT{dc}", name=f"kT{dc}")
                nc.vector.tensor_scalar_add(t[:], in0=ps[:], scalar1=bk_c[dc][:])
                kT.append(t)

            # ---------- v projection + pair-tile assembly -----------------
            for mc in range(NMB):
                ps = psum.tile([P, D], f32, tag="ps", name=f"v_ps{mc}")
                for cc in range(4):
                    nc.tensor.matmul(
                        ps[:], lhsT=augT[cc][:, mc * P:(mc + 1) * P],
                        rhs=wvT[cc][:], start=(cc == 0), stop=False,
                    )
                nc.tensor.matmul(
                    ps[:], lhsT=onesrow[:], rhs=bvr[:], start=False, stop=True,
                )
                for p in range(4):
                    nc.vector.tensor_copy(
                        va[mc][p][:, HD:2 * HD], ps[:, 64 * p:64 * p + HD])
                    nc.vector.tensor_copy(
                        va[mc][p][:, 224:256], ps[:, 64 * p + HD:64 * p + 64])

            # ---------- obs2 stats (feature-major LN via ones-matmuls) ----
            gsq = []
            for oc in range(2):
                t = tmp.tile([P, L], bf16, tag="tmp", name=f"gsq{oc}")
                nc.vector.tensor_mul(t[:], g_t[oc][:], g_t[oc][:])
                gsq.append(t)
            rowsA = consts.tile([P, L], f32, tag="rowsA", name="rowsA")
            rowsB = consts.tile([P, L], f32, tag="rowsB", name="rowsB")
            rowsC = consts.tile([1, L], f32, tag="rowsC", name="rowsC")
            r_mg, r_mg2, r_ug, r_lnug = (rowsA[o:o + 1, :] for o in (0, 32, 64, 96))
            r_rg, r_m1sq, r_u1, r_lnu1 = (rowsB[o:o + 1, :] for o in (0, 32, 64, 96))
            r_r1 = rowsC[0:1, :]

            mg_ps = psum.tile([1, L], f32, tag="ps", name="mg_ps")
            for oc in range(2):
                for nb in range(2):
                    nc.tensor.matmul(
                        mg_ps[:, nb * DD:(nb + 1) * DD],
                        lhsT=inv256c[:], rhs=g_t[oc][:, nb * DD:(nb + 1) * DD],
                        start=(oc == 0), stop=(oc == 1),
                    )
            nc.vector.tensor_copy(r_mg, mg_ps[:])
            sg_ps = psum.tile([1, L], f32, tag="ps", name="sg_ps")
            for oc in range(2):
                for nb in range(2):
                    nc.tensor.matmul(
                        sg_ps[:, nb * DD:(nb + 1) * DD],
                        lhsT=inv256c[:], rhs=gsq[oc][:, nb * DD:(nb + 1) * DD],
                        start=(oc == 0), stop=(oc == 1),
                    )
            nc.vector.tensor_mul(r_mg2, r_mg, r_mg)
            nc.vector.tensor_tensor(out=r_ug, in0=sg_ps[:], in1=r_mg2, op=ALU.subtract)
            nc.scalar.activation(r_lnug, r_ug, AF.Ln, bias=eps_t[64:65, :])
            nc.scalar.activation(r_rg, r_lnug, AF.Exp, scale=-0.5)
            # partition-broadcast via DRAM roundtrip
            nc.sync.dma_start(out=mg_d[:], in_=r_mg)
            nc.sync.dma_start(out=rg_d[:], in_=r_rg)
            mB = consts.tile([P, L], f32, tag="mB", name="mB")
            nc.sync.dma_start(out=mB[:], in_=dram_bcast(mg_d[0:1, :], P))
            rgB = consts.tile([P, L], f32, tag="rgB", name="rgB")
            nc.sync.dma_start(out=rgB[:], in_=dram_bcast(rg_d[0:1, :], P))

            # ---------- obs2 apply -> z chunks 2,3 ------------------------
            z_t = [consts.tile([P, L], bf16, tag=f"z{c}", name=f"z{c}") for c in range(4)]
            for oc in range(2):
                t = tmp.tile([P, L], bf16, tag="tmp", name=f"ot{oc}")
                nc.gpsimd.tensor_tensor(out=t[:], in0=g_t[oc][:], in1=mB[:], op=ALU.subtract)
                t2 = tmp.tile([P, L], bf16, tag="tmp", name=f"ot2{oc}")
                nc.gpsimd.tensor_tensor(out=t2[:], in0=t[:], in1=rgB[:], op=ALU.mult)
                nc.vector.tensor_scalar(
                    out=z_t[2 + oc][:], in0=t2[:],
                    scalar1=gobs_c[oc][:], scalar2=bobsln_c[oc][:],
                    op0=ALU.mult, op1=ALU.add,
                )

            # ---------- attention -----------------------------------------
            e_tiles = {}
            yu_tiles = {}

            def attn_v(grp, mc):
                for pl in range(2):
                    pr = 2 * grp + pl
                    if mc == 0:
                        yu_tiles[pr] = psumy.tile([P, L], f32, tag="yu", name=f"yu{pr}")
                    yt = yu_tiles[pr]
                    ea = e_tiles[(grp, mc)][2 * pl]
                    eb = e_tiles[(grp, mc)][2 * pl + 1]
                    for nb in range(2):
                        sl = slice(nb * DD, (nb + 1) * DD)
                        nc.tensor.matmul(
                            yt[:, sl], lhsT=va[mc][pr][:, 0:P], rhs=ea[:, sl],
                            start=(mc == 0), stop=False,
                        )
                        nc.tensor.matmul(
                            yt[:, sl], lhsT=va[mc][pr][:, P:2 * P], rhs=eb[:, sl],
                            start=False, stop=(mc == NMB - 1),
                        )

            def norm_pair(pr):
                yt = yu_tiles[pr]
                rrow = rrp.tile([P, L], f32, tag="rrow", name=f"rrow{pr}")
                nc.vector.reciprocal_approx_fast(
                    out=rrow[0:2, :], in_=yt[0:2, :])
                nc.sync.dma_start(out=rden_d[pr][0:1, :], in_=rrow[0:1, :])
                nc.sync.dma_start(out=rden_d[pr][1:2, :], in_=rrow[1:2, :])
                rB = rbp.tile([P, L], f32, tag="rB", name=f"rB{pr}")
                nc.sync.dma_start(out=rB[0:96, :], in_=dram_bcast(rden_d[pr][0:1, :], 96))
                nc.sync.dma_start(out=rB[96:128, :], in_=dram_bcast(rden_d[pr][1:2, :], HD))
                st = stg.tile([P, L], bf16, tag="stage", name=f"stage{pr}")
                nc.vector.tensor_tensor(
                    out=st[:, :], in0=yt[:, :], in1=rB[:, :], op=ALU.mult)
                zc = z_t[pr // 2]
                if pr % 2 == 0:
                    nc.sync.dma_start(out=zc[HD:64, :], in_=st[HD:64, :])
                    nc.sync.dma_start(out=zc[96:128, :], in_=st[96:128, :])
                else:
                    nc.sync.dma_start(out=zc[0:HD, :], in_=st[HD:64, :])
                    nc.sync.dma_start(out=zc[64:96, :], in_=st[96:128, :])

            for grp in range(2):
                dc = grp
                for mb in range(NMB):
                    etl = []
                    for hl in range(4):
                        ps = psum.tile([P, L], f32, tag="ps", name=f"sc{grp}_{mb}_{hl}")
                        ro = hl * HD
                        for nb in range(2):
                            nc.tensor.matmul(
                                ps[:, nb * DD:(nb + 1) * DD],
                                lhsT=kT[dc][ro:ro + HD, mb * P:(mb + 1) * P],
                                rhs=qT[dc][ro:ro + HD, nb * DD:(nb + 1) * DD],
                                start=True, stop=True,
                                tile_position=(ro, 0),
                            )
                        et = epool.tile([P, L], bf16, tag="e", name=f"e{grp}_{mb}_{hl}")
                        nc.scalar.activation(et[:], ps[:], AF.Exp)
                        idx = (grp * NMB + mb) * 4 + hl
                        eng = nc.gpsimd if idx % 3 == 0 else nc.vector
                        eng.tensor_tensor(
                            out=et[:], in0=et[:], in1=maskT[mb][:], op=ALU.mult)
                        etl.append(et)
                    e_tiles[(grp, mb)] = etl
                    if grp == 1:
                        attn_v(0, mb)
                        if mb == NMB - 1:
                            norm_pair(0)
                            norm_pair(1)
            for mc in range(NMB):
                attn_v(1, mc)
            norm_pair(2)
            norm_pair(3)

            # ---------- LN1 stats ----------------------------------------
            m1_bf_t = consts.tile([1, L], bf16, tag="m1_bf", name="m1_bf")
            m1_bf = m1_bf_t[0:1, :]
            invr1_t = consts.tile([1, L], bf16, tag="invr1", name="invr1")
            invr1 = invr1_t[0:1, :]
            m1_ps = psum.tile([1, L], f32, tag="ps", name="m1_ps")
            for c in range(4):
                for nb in range(2):
                    nc.tensor.matmul(
                        m1_ps[:, nb * DD:(nb + 1) * DD],
                        lhsT=inv512c[:], rhs=z_t[c][:, nb * DD:(nb + 1) * DD],
                        start=(c == 0), stop=(c == 3),
                    )
            nc.vector.tensor_copy(m1_bf, m1_ps[:])
            s1_ps = psum.tile([1, L], f32, tag="ps", name="s1_ps")
            for c in range(4):
                zq = zsqp.tile([P, L], bf16, tag="zsq", name=f"zsq{c}")
                nc.vector.tensor_mul(zq[:], z_t[c][:], z_t[c][:])
                for nb in range(2):
                    nc.tensor.matmul(
                        s1_ps[:, nb * DD:(nb + 1) * DD],
                        lhsT=inv512c[:], rhs=zq[:, nb * DD:(nb + 1) * DD],
                        start=(c == 0), stop=(c == 3),
                    )
            nc.vector.tensor_tensor(out=r_m1sq, in0=m1_ps[:], in1=m1_bf, op=ALU.mult)
            nc.vector.tensor_tensor(out=r_u1, in0=s1_ps[:], in1=r_m1sq, op=ALU.subtract)
            nc.scalar.activation(r_lnu1, r_u1, AF.Ln, bias=eps_t[64:65, :])
            nc.scalar.activation(r_r1, r_lnu1, AF.Exp, scale=-0.5)
            nc.scalar.activation(invr1, r_lnu1, AF.Exp, scale=0.5)
            # r1 column tile [128, 8] via DRAM roundtrip
            nc.sync.dma_start(out=r1_d[:], in_=r_r1)
            r1col = consts.tile([P, NLB], f32, tag="r1col", name="r1col")
            nc.sync.dma_start(
                out=r1col[:],
                in_=r1_d[:].rearrange("a (b p) -> a p b", p=P),
            )

            # ---------- projection + gelu(scale=rstd1) --------------------
            q2_t = []
            for lb in range(NLB):
                ps = psum.tile([P, D], f32, tag="ps", name=f"p_ps{lb}")
                for cc in range(4):
                    nc.tensor.matmul(
                        ps[:], lhsT=z_t[cc][:, lb * P:(lb + 1) * P],
                        rhs=w1T[cc][:], start=(cc == 0), stop=False,
                    )
                nc.tensor.matmul(
                    ps[:], lhsT=m1_bf[:, lb * P:(lb + 1) * P], rhs=negc1[:],
                    start=False, stop=False,
                )
                nc.tensor.matmul(
                    ps[:], lhsT=invr1[:, lb * P:(lb + 1) * P], rhs=c2r[:],
                    start=False, stop=True,
                )
                qt = q2p.tile([P, D], bf16, tag="q2", name=f"q2_{lb}")
                nc.scalar.activation(qt[:], ps[:], AF.Gelu, scale=r1col[:, lb:lb + 1])
                q2_t.append(qt)

            # ---------- LN2 + output --------------------------------------
            mv2 = consts.tile([P, 2 * NLB], f32, tag="mv2", name="mv2")
            for lb in range(NLB):
                st = small.tile([P, nc.vector.BN_STATS_DIM], f32, tag="st", name=f"st{lb}")
                nc.vector.bn_stats(out=st[:], in_=q2_t[lb][:])
                nc.vector.bn_aggr(out=mv2[:, 2 * lb:2 * lb + 2], in_=st[:])
            lnv2 = consts.tile([P, NLB], f32, tag="lnv2", name="lnv2")
            nc.scalar.activation(
                lnv2[:].rearrange("p (b one) -> p b one", one=1),
                mv2[:].rearrange("p (b two) -> p b two", two=2)[:, :, 1:2],
                AF.Ln, bias=eps_t[:],
            )
            r2 = consts.tile([P, NLB], f32, tag="r2", name="r2")
            nc.scalar.activation(r2[:], lnv2[:], AF.Exp, scale=-0.5)
            for lb in range(NLB):
                t = small.tile([P, D], bf16, tag="t2", name=f"t2_{lb}")
                nc.vector.tensor_scalar(
                    out=t[:], in0=q2_t[lb][:],
                    scalar1=mv2[:, 2 * lb:2 * lb + 1], scalar2=r2[:, lb:lb + 1],
                    op0=ALU.subtract, op1=ALU.mult,
                )
                t3 = small.tile([P, D], bf16, tag="t3", name=f"t3_{lb}")
                nc.vector.tensor_mul(t3[:], t[:], g2b[:])
                ot = outp.tile([P, D], f32, tag="outt", name=f"outt{lb}")
                nc.vector.tensor_add(ot[:], t3[:], b2b[:])
                nc.sync.dma_start(out=out[lb * P:(lb + 1) * P, :], in_=ot[:])

    nc.compile()
    return nc


def get_nc(body_reps=1):
    if "nc" not in _CACHE:
        _CACHE["nc"] = _build()
    return _CACHE["nc"]


def make_in_maps(inputs):
    import ml_dtypes

    bf = ml_dtypes.bfloat16
    B = inputs["observations"].shape[0]
    Wp = np.asarray(inputs["Wp"], np.float64)
    g1 = np.asarray(inputs["g1"], np.float64)
    b1 = np.asarray(inputs["b1"], np.float64)
    W1 = Wp * g1[None, :]  # (256, 512)
    yperm = np.concatenate([np.arange(32 * h, 32 * h + 32) for h in Y_HEAD_ORDER])
    W1p = np.concatenate([W1[:, :D][:, yperm], W1[:, D:]], axis=1)
    c1 = Wp @ g1
    c2 = Wp @ b1

    vsk = np.zeros((P, 2 * P), np.float64)
    vsk[:, HD] = 1.0
    vsk[:, 224] = 1.0
    shared = {
        "wq_bf": np.ascontiguousarray(inputs["Wq"], dtype=bf),
        "wk_bf": np.ascontiguousarray(inputs["Wk"], dtype=bf),
        "wv_bf": np.ascontiguousarray(inputs["Wv"], dtype=bf),
        "wobs_bf": np.ascontiguousarray(inputs["Wobs"], dtype=bf),
        "w1t_bf": np.ascontiguousarray(W1p.T, dtype=bf),
        "negc1_row": np.ascontiguousarray(-c1[None, :], dtype=bf),
        "c2_row": np.ascontiguousarray(c2[None, :], dtype=bf),
        "bv_row": np.ascontiguousarray(np.asarray(inputs["bv"])[None, :], dtype=bf),
    }
    for k in ("bq", "bk", "bobs", "g_obs", "b_obs", "g2", "b2"):
        shared[k] = np.ascontiguousarray(inputs[k], dtype=np.float32)
    in_maps = []
    for b in range(B):
        m = dict(shared)
        m["obs_bf"] = np.ascontiguousarray(inputs["observations"][b], dtype=bf)
        m["act_bf"] = np.ascontiguousarray(inputs["actions"][b], dtype=bf)
        am = (np.asarray(inputs["atten_masks"][b]) != 0).astype(np.float32)
        np.fill_diagonal(am, 0.0)
        m["msk_bf"] = np.ascontiguousarray(am, dtype=bf)
        in_maps.append(m)
    return in_maps


def kernel(**inputs):
    from concourse.bass_utils import run_bass_kernel_spmd

    nc = get_nc()
    in_maps = make_in_maps(inputs)
    res = run_bass_kernel_spmd(nc, in_maps, list(range(NCORES)))
    return np.stack([r["out"] for r in res.results], axis=0)
